# revision 67
# baseline (speedup 1.0000x reference)
"""Trainium2 Bass kernel for nn_MultiHeadAttention_64733747085699.

Sharding: tensor-parallel over heads (4 heads / core) x data-parallel over
batch (2 batches / core) across 8 NeuronCores. Each core computes a partial
output projection for its 4 heads; the host sums the 4 TP partials per batch
group and adds the output bias.

Per-core device pipeline (all matmuls on PE, fp32 PSUM accumulation):
  xT  = transpose-load of x (bf16, via DMA xbar transpose)
  qT/kT [e,s] = W @ xT   (fp32 out, per-partition bias fused in evacuation)
  v [s,e]    = xT.T @ Wv (bf16, bias via broadcast tile add)
  scoresT[sk,sq] = k^T q / sqrt(d), two heads row-packed on the PE array
  expT = exp(scoresT) on ACT, causal mask applied as a 0/1 multiply on DVE
  attn_outT[d,sq] = v^T expT, two heads column-packed; den via ones-matmul
  out[s,e] partial = attn_outT.T @ WoT with per-column normalization folded
  into a single DVE divide during PSUM evacuation.
"""
import sys

if "/opt/trn_rl_repo" not in sys.path:
    sys.path.insert(0, "/opt/trn_rl_repo")

import numpy as np
import ml_dtypes

import bass_rust
import concourse.bass as bass
import concourse.tile as tile
from concourse import mybir
from concourse import library_config
from concourse.vector_clock import ScopedClock

BF16 = ml_dtypes.bfloat16
F32 = np.float32

D_MODEL = 1024
N_HEADS = 16
D_HEAD = 64
B, S = 4, 2048
N_CORES = 8
TP = 4          # head-parallel ranks
DP = 2          # batch-parallel groups
E = D_MODEL // TP        # 256 e-channels per core (4 heads)
SL = 2 * S               # 4096 core-local sequence rows (2 batches)
NK = D_MODEL // 128      # 8 contraction tiles
NSLAB = E // 128         # 2 e-slabs per core (2 heads each)
NST = SL // 128          # 32 local s-tiles
NSC = SL // 512          # 8 local s-chunks
SCALE = 1.0 / np.sqrt(D_HEAD)

fp32 = mybir.dt.float32
bf16 = mybir.dt.bfloat16


# ---------------------------------------------------------------------------
# Workaround: the pinned walrus codegen rejects instructions carrying more
# than one sync-wait command. After Tile scheduling, hoist extra waits onto
# same-engine NOPs inserted immediately before the offending instruction
# (semantically identical: the waits still complete before it executes).
def _split_multi_waits(nc: bass.Bass) -> None:
    for _, bbc in nc.bb_map.items():
        bb = bbc.bb
        insts = bb.instructions
        new_list = []
        changed = False
        for inst in insts:
            try:
                si = inst.sync_info
                waits = list(si.on_wait)
            except Exception:
                new_list.append(inst)
                continue
            if len(waits) > 1:
                changed = True
                for w in waits[:-1]:
                    nop = mybir.InstNoOp(
                        name=nc.get_next_instruction_name(), ins=[], outs=[]
                    )
                    nop.engine = inst.engine
                    nop.sync_info = bass_rust.SyncInfo(on_wait=[w], on_update=[])
                    nc.register_instruction(nop, overwrite=True)
                    new_list.append(nop)
                inst.sync_info = bass_rust.SyncInfo(
                    on_wait=[waits[-1]], on_update=list(si.on_update)
                )
            new_list.append(inst)
        if changed:
            bb.instructions = new_list
# ---------------------------------------------------------------------------


def build_nc(phases: str = "ABC", trace_sim: bool = False) -> bass.Bass:
    nc = bass.Bass("TRN2", target_bir_lowering=False, debug=False)
    x_bf = nc.dram_tensor("x_bf", [SL, D_MODEL], bf16, kind="ExternalInput")
    wq_t = nc.dram_tensor("wq_t", [D_MODEL, E], bf16, kind="ExternalInput")
    wk_t = nc.dram_tensor("wk_t", [D_MODEL, E], bf16, kind="ExternalInput")
    wv_t = nc.dram_tensor("wv_t", [D_MODEL, E], bf16, kind="ExternalInput")
    wo_t = nc.dram_tensor("wo_t", [E, D_MODEL], bf16, kind="ExternalInput")
    bqs = nc.dram_tensor("bqs", [128, NSLAB], fp32, kind="ExternalInput")
    bks = nc.dram_tensor("bks", [128, NSLAB], fp32, kind="ExternalInput")
    bvs = nc.dram_tensor("bvs", [1, E], fp32, kind="ExternalInput")
    tri4 = nc.dram_tensor("tri4", [128, 4, 512], bf16, kind="ExternalInput")
    out = nc.dram_tensor("out", [SL, D_MODEL], fp32, kind="ExternalOutput")

    with tile.TileContext(nc, trace_sim=trace_sim) as tc:
        with tc.tile_pool(name="persist", bufs=1) as persist:
            # --- persistent SBUF tensors -----------------------------------
            xTp = persist.tile([128, NK, SL], bf16, tag="xT")
            wk_sb = persist.tile([128, NK, E], bf16)
            nc.scalar.dma_start(out=wk_sb, in_=wk_t.rearrange("(k p) e -> p k e", p=128))
            bk_sb = persist.tile([128, NSLAB], fp32)
            nc.scalar.dma_start(out=bk_sb, in_=bks[:, :])
            if "X" in phases or "A" in phases:
                for k in range(NK):
                    nc.sync.dma_start_transpose(
                        out=xTp[:, k, 0 : SL // 2],
                        in_=x_bf[0 : SL // 2, k * 128 : (k + 1) * 128],
                    )
            wq_sb = persist.tile([128, NK, E], bf16)
            nc.scalar.dma_start(out=wq_sb, in_=wq_t.rearrange("(k p) e -> p k e", p=128))
            bq_sb = persist.tile([128, NSLAB], fp32)
            nc.scalar.dma_start(out=bq_sb, in_=bqs[:, :])
            wv_sb = persist.tile([128, NK, E], bf16)
            nc.scalar.dma_start(out=wv_sb, in_=wv_t.rearrange("(k p) e -> p k e", p=128))
            wo_sb = persist.tile([128, NSLAB, D_MODEL], bf16)
            nc.scalar.dma_start(out=wo_sb, in_=wo_t.rearrange("(s p) e -> p s e", p=128))
            tri_sb = persist.tile([128, 4, 512], bf16)
            nc.scalar.dma_start(out=tri_sb, in_=tri4[:, :, :])
            allones = persist.tile([128, D_HEAD], bf16)
            nc.vector.memset(allones, 1.0)
            # per-head bias-v broadcast tiles [128, 64] via partition-stride-0
            # SWDGE DMA from DRAM
            bv_b = persist.tile([128, 4, D_HEAD], fp32)
            for h in range(4):
                seg = bvs[0:1, h * D_HEAD : (h + 1) * D_HEAD]
                src = bass.AP(
                    tensor=seg.tensor,
                    offset=seg.offset,
                    ap=[[0, 128]] + list(seg.ap[1:]),
                )
                nc.gpsimd.dma_start(out=bv_b[:, h, :], in_=src)

            qT = persist.tile([128, NSLAB, SL], mybir.dt.float32r)
            kT = persist.tile([128, NSLAB, SL], mybir.dt.float32r)
            v_sb = persist.tile([128, NST, 4, D_HEAD], bf16)
            aoT = persist.tile([128, NSLAB, SL], bf16)

            # --- phase A: projections --------------------------------------
            with tc.tile_pool(name="pha", bufs=1) as pha, tc.tile_pool(
                name="psA", bufs=2, space="PSUM"
            ) as psA:
                xT = xTp
                if "X" in phases or "A" in phases:
                    lo = SL // 2
                    for k in range(NK):
                        nc.sync.dma_start_transpose(
                            out=xT[:, k, lo:],
                            in_=x_bf[lo:, k * 128 : (k + 1) * 128],
                        )

                # batch-0 projections first so attention can overlap batch-1's
                def emit_qk(wsb, bsb, dest, et, sc):
                    ps = psA.tile([128, 512], fp32, tag="ps_qk")
                    for k in range(NK):
                        nc.tensor.matmul(
                            ps,
                            lhsT=wsb[:, k, et * 128 : (et + 1) * 128],
                            rhs=xT[:, k, sc * 512 : (sc + 1) * 512],
                            start=(k == 0),
                            stop=(k == NK - 1),
                        )
                    nc.vector.tensor_scalar(
                        dest[:, et, sc * 512 : (sc + 1) * 512],
                        ps,
                        bsb[:, et : et + 1],
                        None,
                        mybir.AluOpType.add,
                    )

                def emit_v(st):
                    ps = psA.tile([128, E], fp32, tag="ps_v")
                    for k in range(NK):
                        nc.tensor.matmul(
                            ps,
                            lhsT=xT[:, k, st * 128 : (st + 1) * 128],
                            rhs=wv_sb[:, k, :],
                            start=(k == 0),
                            stop=(k == NK - 1),
                        )
                    for h in range(4):
                        nc.vector.tensor_add(
                            v_sb[:, st, h, :],
                            ps[:, h * D_HEAD : (h + 1) * D_HEAD],
                            bv_b[:, h, :],
                        )

                # batch-0 (half 0) first so its attention can overlap the
                # rest; within a half: slab-0 k/q, then v, then slab-1 k/q
                for half in range(2 if "A" in phases else 0):
                    scs = range(half * NSC // 2, (half + 1) * NSC // 2)
                    sts = range(half * NST // 2, (half + 1) * NST // 2)
                    for sc in scs:
                        emit_qk(wk_sb, bk_sb, kT, 0, sc)
                    for sc in scs:
                        emit_qk(wq_sb, bq_sb, qT, 0, sc)
                    for st in sts:
                        emit_v(st)
                    for sc in scs:
                        emit_qk(wk_sb, bk_sb, kT, 1, sc)
                    for sc in scs:
                        emit_qk(wq_sb, bq_sb, qT, 1, sc)

            # --- phase B: attention ----------------------------------------
            with tc.tile_pool(name="phb", bufs=6) as phb, tc.tile_pool(
                name="phd", bufs=2
            ) as phd, tc.tile_pool(name="psS", bufs=2, space="PSUM") as psS, tc.tile_pool(
                name="psAV", bufs=2, space="PSUM"
            ) as psAV, tc.tile_pool(name="psDen", bufs=2, space="PSUM") as psDen:
                for b in range(DP if "B" in phases else 0):
                    for slab in range(NSLAB):
                        for c in range(4):
                            sq0 = 2048 * b + 512 * c
                            T = 4 * c + 4
                            ps_av = psAV.tile([128, 512], fp32, tag="av")
                            den_ps = psDen.tile([128, 512], fp32, tag="dn")
                            def emit_scores_exp(t):
                                sk0 = 2048 * b + 128 * t
                                ps_s = psS.tile([128, 2, 512], fp32, tag="sc")
                                for h in range(2):
                                    nc.tensor.matmul(
                                        ps_s[:, h, :],
                                        lhsT=kT[
                                            64 * h : 64 * (h + 1),
                                            slab,
                                            sk0 : sk0 + 128,
                                        ],
                                        rhs=qT[
                                            64 * h : 64 * (h + 1), slab, sq0 : sq0 + 512
                                        ],
                                        start=True,
                                        stop=True,
                                        tile_position=(64 * h, 0),
                                    )
                                # columns < 128j are fully masked for this
                                # sk-tile: skip them in exp and downstream mms
                                j = t - 4 * c
                                lo = 128 * j if j > 0 else 0
                                ex = phb.tile([128, 2, 512], bf16, tag="ex")
                                nc.scalar.activation(
                                    out=ex[:, :, lo:],
                                    in_=ps_s[:, :, lo:],
                                    func=mybir.ActivationFunctionType.Exp,
                                    scale=float(SCALE),
                                )
                                if j >= 0:
                                    for h in range(2):
                                        nc.gpsimd.tensor_mul(
                                            ex[:, h, 128 * j : 128 * j + 128],
                                            ex[:, h, 128 * j : 128 * j + 128],
                                            tri_sb[:, j, 128 * j : 128 * j + 128],
                                        )
                                return ex, lo

                            def emit_av(t, ex, lo):
                                st_g = 16 * b + t
                                for h in range(2):
                                    hg = 2 * slab + h
                                    nc.tensor.matmul(
                                        ps_av[64 * h : 64 * (h + 1), lo:],
                                        lhsT=v_sb[:, st_g, hg, :],
                                        rhs=ex[:, h, lo:],
                                        start=(t == 0),
                                        stop=(t == T - 1),
                                        tile_position=(0, 64 * h),
                                        skip_group_check=True,
                                    )
                                    nc.tensor.matmul(
                                        den_ps[64 * h : 64 * (h + 1), lo:],
                                        lhsT=allones,
                                        rhs=ex[:, h, lo:],
                                        start=(t == 0),
                                        stop=(t == T - 1),
                                        tile_position=(0, 64 * h),
                                        skip_group_check=True,
                                    )

                            # lookahead-1 software pipeline: scores(t+1) is
                            # emitted (and runs on PE) while exp(t) is on ACT
                            # lookahead-2 software pipeline
                            pend = []
                            for t in range(T):
                                pend.append((t, *emit_scores_exp(t)))
                                if len(pend) > 3:
                                    emit_av(*pend.pop(0))
                            for item in pend:
                                emit_av(*item)
                            den_rb = phd.tile([128, 512], fp32, tag="den_rb")
                            nc.vector.reciprocal(den_rb, den_ps)
                            nc.vector.tensor_mul(
                                aoT[:, slab, sq0 : sq0 + 512], ps_av, den_rb
                            )

            # --- phase C: output projection --------------------------------
            with tc.tile_pool(name="phc", bufs=3) as phc, tc.tile_pool(
                name="psC", bufs=2, space="PSUM"
            ) as psC:
                for st in range(NST if "C" in phases else 0):
                    ot = phc.tile([128, D_MODEL], fp32, tag="ot")
                    for eh in range(2):
                        ps = psC.tile([128, 512], fp32, tag="ps_o")
                        for s in range(NSLAB):
                            nc.tensor.matmul(
                                ps,
                                lhsT=aoT[:, s, st * 128 : (st + 1) * 128],
                                rhs=wo_sb[:, s, eh * 512 : (eh + 1) * 512],
                                start=(s == 0),
                                stop=(s == NSLAB - 1),
                            )
                        nc.vector.tensor_copy(ot[:, eh * 512 : (eh + 1) * 512], ps)
                    nc.sync.dma_start(
                        out=out[st * 128 : (st + 1) * 128, :], in_=ot
                    )
    _split_multi_waits(nc)
    return nc


def make_core_inputs(x, Wq, bq, Wk, bk, Wv, bv, Wo):
    """Host-side shard prep: returns list of 8 per-core input dicts."""
    x = np.asarray(x, F32)
    tri4 = np.zeros((128, 4, 512), BF16)
    for p in range(128):
        for j in range(4):
            tri4[p, j, 128 * j + p :] = 1.0
    ins = []
    for c in range(N_CORES):
        dp, tp = divmod(c, TP)
        es = slice(E * tp, E * (tp + 1))
        ins.append(
            {
                "x_bf": np.ascontiguousarray(
                    x[2 * dp : 2 * dp + 2].reshape(SL, D_MODEL)
                ).astype(BF16),
                "wq_t": np.ascontiguousarray(np.asarray(Wq, F32)[es, :].T).astype(BF16),
                "wk_t": np.ascontiguousarray(np.asarray(Wk, F32)[es, :].T).astype(BF16),
                "wv_t": np.ascontiguousarray(np.asarray(Wv, F32)[es, :].T).astype(BF16),
                "wo_t": np.ascontiguousarray(np.asarray(Wo, F32)[:, es].T).astype(BF16),
                "bqs": np.ascontiguousarray(
                    np.asarray(bq, F32)[es].reshape(NSLAB, 128).T
                ),
                "bks": np.ascontiguousarray(
                    np.asarray(bk, F32)[es].reshape(NSLAB, 128).T
                ),
                "bvs": np.ascontiguousarray(np.asarray(bv, F32)[es].reshape(1, E)),
                "tri4": tri4,
            }
        )
    return ins


def assemble_output(results, bo):
    """Sum TP partials per DP group, add output bias."""
    out = np.zeros((B, S, D_MODEL), F32)
    for dp in range(DP):
        acc = np.zeros((SL, D_MODEL), F32)
        for tp in range(TP):
            acc += results[dp * TP + tp]["out"]
        out[2 * dp : 2 * dp + 2] = acc.reshape(2, S, D_MODEL)
    return out + np.asarray(bo, F32)[None, None, :]


_EXEC_CACHE: dict = {}


def _build_exec(donate: bool):
    import jax
    from jax.experimental.shard_map import shard_map
    from jax.sharding import Mesh, PartitionSpec
    from concourse import bass2jax

    if "nc" in _EXEC_CACHE:
        nc = _EXEC_CACHE["nc"]
    else:
        nc = build_nc()
        _EXEC_CACHE["nc"] = nc
    bass2jax.install_neuronx_cc_hook()

    partition_name = nc.partition_id_tensor.name if nc.partition_id_tensor else None
    in_names, out_names, out_avals, zero_outs = [], [], [], []
    for alloc in nc.m.functions[0].allocations:
        if not isinstance(alloc, mybir.MemoryLocationSet):
            continue
        name = alloc.memorylocations[0].name
        if alloc.kind == "ExternalInput":
            if name != partition_name:
                in_names.append(name)
        elif alloc.kind == "ExternalOutput":
            out_names.append(name)
            shape = tuple(alloc.tensor_shape)
            dtype = mybir.dt.np(alloc.dtype)
            out_avals.append(jax.core.ShapedArray(shape, dtype))
            zero_outs.append(np.zeros(shape, dtype))
    n_params = len(in_names)
    n_outs = len(out_avals)
    all_names = in_names + out_names
    if partition_name is not None:
        all_names = all_names + [partition_name]

    def _body(*args):
        operands = list(args)
        if partition_name is not None:
            operands.append(bass2jax.partition_id_tensor())
        outs = bass2jax._bass_exec_p.bind(
            *operands,
            out_avals=tuple(out_avals),
            in_names=tuple(all_names),
            out_names=tuple(out_names),
            lowering_input_output_aliases=(),
            sim_require_finite=True,
            sim_require_nnan=True,
            nc=nc,
        )
        return tuple(outs)

    devices = jax.devices()[:N_CORES]
    mesh = Mesh(np.asarray(devices), ("core",))
    donate_nums = tuple(range(n_params, n_params + n_outs)) if donate else ()
    sharded = jax.jit(
        shard_map(
            _body,
            mesh=mesh,
            in_specs=(PartitionSpec("core"),) * (n_params + n_outs),
            out_specs=(PartitionSpec("core"),) * n_outs,
            check_rep=False,
        ),
        donate_argnums=donate_nums,
        keep_unused=True,
    )
    meta = (in_names, out_names, out_avals, zero_outs, n_params)
    return sharded, meta


def _get_runner():
    """Build (once) a cached jitted SPMD runner over 8 cores."""
    if "run" in _EXEC_CACHE:
        return _EXEC_CACHE["run"]

    sharded, meta = _build_exec(donate=True)
    in_names, out_names, out_avals, zero_outs, n_params = meta

    def run(in_maps):
        concat_in = [
            np.concatenate([np.asarray(m[name]) for m in in_maps], axis=0)
            for name in in_names
        ]
        concat_zeros = [
            np.zeros((N_CORES * z.shape[0], *z.shape[1:]), z.dtype) for z in zero_outs
        ]
        out_arrs = sharded(*concat_in, *concat_zeros)
        return [
            {
                name: np.asarray(out_arrs[i]).reshape(N_CORES, *out_avals[i].shape)[c]
                for i, name in enumerate(out_names)
            }
            for c in range(N_CORES)
        ]

    _EXEC_CACHE["run"] = run
    _EXEC_CACHE["sharded"] = sharded
    _EXEC_CACHE["meta"] = meta
    return run


def kernel(x, mask, Wq, bq, Wk, bk, Wv, bv, Wo, bo):
    run = _get_runner()
    ins = make_core_inputs(x, Wq, bq, Wk, bk, Wv, bv, Wo)
    results = run(ins)
    return assemble_output(results, bo)


# revision 69
# speedup vs baseline: 37.8824x; 37.8824x over previous
"""Trainium2 Bass kernel for nn_MultiHeadAttention_64733747085699.

Sharding: tensor-parallel over heads (4 heads / core) x data-parallel over
batch (2 batches / core) across 8 NeuronCores. Each core computes a partial
output projection for its 4 heads; the host sums the 4 TP partials per batch
group and adds the output bias.

Per-core device pipeline (all matmuls on PE, fp32 PSUM accumulation):
  xT  = transpose-load of x (bf16, via DMA xbar transpose)
  qT/kT [e,s] = W @ xT   (float32r out, per-partition bias in evacuation)
  v [s,e]    = xT.T @ Wv (bf16, bias via broadcast tile add)
  scoresT[sk,sq] = k^T q, f32r matmuls, two heads row-packed on the PE array
  expT = exp(scoresT/sqrt(d)) on ACT (scale fused), restricted to the causal
  column range; triangle mask applied as a 0/1 multiply on GpSimd
  attn_outT[d,sq] = v^T expT, two heads column-packed, lookahead-3 software
  pipeline against the ACT exp stream; softmax denominators accumulated
  partition-broadcast via an all-ones stationary matrix in the same pass
  normalization = DVE reciprocal + multiply during PSUM evacuation
  out[s,e] partial = attn_outT.T @ WoT
Host: bf16 cast + weight transposes + TP-partial reduction + biases.
"""
import sys

if "/opt/trn_rl_repo" not in sys.path:
    sys.path.insert(0, "/opt/trn_rl_repo")

import numpy as np
import ml_dtypes

import bass_rust
import concourse.bass as bass
import concourse.tile as tile
from concourse import mybir

BF16 = ml_dtypes.bfloat16
F32 = np.float32

D_MODEL = 1024
N_HEADS = 16
D_HEAD = 64
B, S = 4, 2048
N_CORES = 8
TP = 4          # head-parallel ranks
DP = 2          # batch-parallel groups
E = D_MODEL // TP        # 256 e-channels per core (4 heads)
SL = 2 * S               # 4096 core-local sequence rows (2 batches)
NK = D_MODEL // 128      # 8 contraction tiles
NSLAB = E // 128         # 2 e-slabs per core (2 heads each)
NST = SL // 128          # 32 local s-tiles
NSC = SL // 512          # 8 local s-chunks
SCALE = 1.0 / np.sqrt(D_HEAD)

fp32 = mybir.dt.float32
bf16 = mybir.dt.bfloat16


# ---------------------------------------------------------------------------
# Workaround: the pinned walrus codegen rejects instructions carrying more
# than one sync-wait command. After Tile scheduling, hoist extra waits onto
# same-engine NOPs inserted immediately before the offending instruction
# (semantically identical: the waits still complete before it executes).
def _split_multi_waits(nc: bass.Bass) -> None:
    for _, bbc in nc.bb_map.items():
        bb = bbc.bb
        insts = bb.instructions
        new_list = []
        changed = False
        for inst in insts:
            try:
                si = inst.sync_info
                waits = list(si.on_wait)
            except Exception:
                new_list.append(inst)
                continue
            if len(waits) > 1:
                changed = True
                for w in waits[:-1]:
                    nop = mybir.InstNoOp(
                        name=nc.get_next_instruction_name(), ins=[], outs=[]
                    )
                    nop.engine = inst.engine
                    nop.sync_info = bass_rust.SyncInfo(on_wait=[w], on_update=[])
                    nc.register_instruction(nop, overwrite=True)
                    new_list.append(nop)
                inst.sync_info = bass_rust.SyncInfo(
                    on_wait=[waits[-1]], on_update=list(si.on_update)
                )
            new_list.append(inst)
        if changed:
            bb.instructions = new_list
# ---------------------------------------------------------------------------


def build_nc(phases: str = "ABC", trace_sim: bool = False) -> bass.Bass:
    nc = bass.Bass("TRN2", target_bir_lowering=False, debug=False)
    x_bf = nc.dram_tensor("x_bf", [SL, D_MODEL], bf16, kind="ExternalInput")
    wq_t = nc.dram_tensor("wq_t", [D_MODEL, E], bf16, kind="ExternalInput")
    wk_t = nc.dram_tensor("wk_t", [D_MODEL, E], bf16, kind="ExternalInput")
    wv_t = nc.dram_tensor("wv_t", [D_MODEL, E], bf16, kind="ExternalInput")
    wo_t = nc.dram_tensor("wo_t", [E, D_MODEL], bf16, kind="ExternalInput")
    bqs = nc.dram_tensor("bqs", [128, NSLAB], fp32, kind="ExternalInput")
    bks = nc.dram_tensor("bks", [128, NSLAB], fp32, kind="ExternalInput")
    bvs = nc.dram_tensor("bvs", [1, E], fp32, kind="ExternalInput")
    tri4 = nc.dram_tensor("tri4", [128, 4, 512], bf16, kind="ExternalInput")
    out = nc.dram_tensor("out", [SL, D_MODEL], fp32, kind="ExternalOutput")

    with tile.TileContext(nc, trace_sim=trace_sim) as tc:
        with tc.tile_pool(name="persist", bufs=1) as persist:
            # --- persistent SBUF tensors -----------------------------------
            xTp = persist.tile([128, NK, SL], bf16, tag="xT")
            wk_sb = persist.tile([128, NK, E], bf16)
            nc.scalar.dma_start(out=wk_sb, in_=wk_t.rearrange("(k p) e -> p k e", p=128))
            bk_sb = persist.tile([128, NSLAB], fp32)
            nc.scalar.dma_start(out=bk_sb, in_=bks[:, :])
            if "X" in phases or "A" in phases:
                for k in range(NK):
                    nc.sync.dma_start_transpose(
                        out=xTp[:, k, 0 : SL // 2],
                        in_=x_bf[0 : SL // 2, k * 128 : (k + 1) * 128],
                    )
            wq_sb = persist.tile([128, NK, E], bf16)
            nc.scalar.dma_start(out=wq_sb, in_=wq_t.rearrange("(k p) e -> p k e", p=128))
            bq_sb = persist.tile([128, NSLAB], fp32)
            nc.scalar.dma_start(out=bq_sb, in_=bqs[:, :])
            wv_sb = persist.tile([128, NK, E], bf16)
            nc.scalar.dma_start(out=wv_sb, in_=wv_t.rearrange("(k p) e -> p k e", p=128))
            wo_sb = persist.tile([128, NSLAB, D_MODEL], bf16)
            nc.scalar.dma_start(out=wo_sb, in_=wo_t.rearrange("(s p) e -> p s e", p=128))
            tri_sb = persist.tile([128, 4, 512], bf16)
            nc.scalar.dma_start(out=tri_sb, in_=tri4[:, :, :])
            allones = persist.tile([128, D_HEAD], bf16)
            nc.vector.memset(allones, 1.0)
            # per-head bias-v broadcast tiles [128, 64] via partition-stride-0
            # SWDGE DMA from DRAM
            bv_b = persist.tile([128, 4, D_HEAD], fp32)
            for h in range(4):
                seg = bvs[0:1, h * D_HEAD : (h + 1) * D_HEAD]
                src = bass.AP(
                    tensor=seg.tensor,
                    offset=seg.offset,
                    ap=[[0, 128]] + list(seg.ap[1:]),
                )
                nc.gpsimd.dma_start(out=bv_b[:, h, :], in_=src)

            qT = persist.tile([128, NSLAB, SL], mybir.dt.float32r)
            kT = persist.tile([128, NSLAB, SL], mybir.dt.float32r)
            v_sb = persist.tile([128, NST, 4, D_HEAD], bf16)
            aoT = persist.tile([128, NSLAB, SL], bf16)

            # --- phase A: projections --------------------------------------
            with tc.tile_pool(name="pha", bufs=1) as pha, tc.tile_pool(
                name="psA", bufs=2, space="PSUM"
            ) as psA:
                xT = xTp
                if "X" in phases or "A" in phases:
                    lo = SL // 2
                    for k in range(NK):
                        nc.sync.dma_start_transpose(
                            out=xT[:, k, lo:],
                            in_=x_bf[lo:, k * 128 : (k + 1) * 128],
                        )

                # batch-0 projections first so attention can overlap batch-1's
                def emit_qk(wsb, bsb, dest, et, sc):
                    ps = psA.tile([128, 512], fp32, tag="ps_qk")
                    for k in range(NK):
                        nc.tensor.matmul(
                            ps,
                            lhsT=wsb[:, k, et * 128 : (et + 1) * 128],
                            rhs=xT[:, k, sc * 512 : (sc + 1) * 512],
                            start=(k == 0),
                            stop=(k == NK - 1),
                        )
                    nc.vector.tensor_scalar(
                        dest[:, et, sc * 512 : (sc + 1) * 512],
                        ps,
                        bsb[:, et : et + 1],
                        None,
                        mybir.AluOpType.add,
                    )

                def emit_v(st):
                    ps = psA.tile([128, E], fp32, tag="ps_v")
                    for k in range(NK):
                        nc.tensor.matmul(
                            ps,
                            lhsT=xT[:, k, st * 128 : (st + 1) * 128],
                            rhs=wv_sb[:, k, :],
                            start=(k == 0),
                            stop=(k == NK - 1),
                        )
                    for h in range(4):
                        nc.vector.tensor_add(
                            v_sb[:, st, h, :],
                            ps[:, h * D_HEAD : (h + 1) * D_HEAD],
                            bv_b[:, h, :],
                        )

                # batch-0 (half 0) first so its attention can overlap the
                # rest; within a half: slab-0 k/q, then v, then slab-1 k/q
                for half in range(2 if "A" in phases else 0):
                    scs = range(half * NSC // 2, (half + 1) * NSC // 2)
                    sts = range(half * NST // 2, (half + 1) * NST // 2)
                    for sc in scs:
                        emit_qk(wk_sb, bk_sb, kT, 0, sc)
                    for sc in scs:
                        emit_qk(wq_sb, bq_sb, qT, 0, sc)
                    for st in sts:
                        emit_v(st)
                    for sc in scs:
                        emit_qk(wk_sb, bk_sb, kT, 1, sc)
                    for sc in scs:
                        emit_qk(wq_sb, bq_sb, qT, 1, sc)

            # --- phase B: attention ----------------------------------------
            with tc.tile_pool(name="phb", bufs=6) as phb, tc.tile_pool(
                name="phd", bufs=2
            ) as phd, tc.tile_pool(name="psS", bufs=2, space="PSUM") as psS, tc.tile_pool(
                name="psAV", bufs=2, space="PSUM"
            ) as psAV, tc.tile_pool(name="psDen", bufs=2, space="PSUM") as psDen:
                for b in range(DP if "B" in phases else 0):
                    for slab in range(NSLAB):
                        for c in range(4):
                            sq0 = 2048 * b + 512 * c
                            T = 4 * c + 4
                            ps_av = psAV.tile([128, 512], fp32, tag="av")
                            den_ps = psDen.tile([128, 512], fp32, tag="dn")
                            def emit_scores_exp(t):
                                sk0 = 2048 * b + 128 * t
                                ps_s = psS.tile([128, 2, 512], fp32, tag="sc")
                                for h in range(2):
                                    nc.tensor.matmul(
                                        ps_s[:, h, :],
                                        lhsT=kT[
                                            64 * h : 64 * (h + 1),
                                            slab,
                                            sk0 : sk0 + 128,
                                        ],
                                        rhs=qT[
                                            64 * h : 64 * (h + 1), slab, sq0 : sq0 + 512
                                        ],
                                        start=True,
                                        stop=True,
                                        tile_position=(64 * h, 0),
                                    )
                                # columns < 128j are fully masked for this
                                # sk-tile: skip them in exp and downstream mms
                                j = t - 4 * c
                                lo = 128 * j if j > 0 else 0
                                ex = phb.tile([128, 2, 512], bf16, tag="ex")
                                nc.scalar.activation(
                                    out=ex[:, :, lo:],
                                    in_=ps_s[:, :, lo:],
                                    func=mybir.ActivationFunctionType.Exp,
                                    scale=float(SCALE),
                                )
                                if j >= 0:
                                    for h in range(2):
                                        nc.gpsimd.tensor_mul(
                                            ex[:, h, 128 * j : 128 * j + 128],
                                            ex[:, h, 128 * j : 128 * j + 128],
                                            tri_sb[:, j, 128 * j : 128 * j + 128],
                                        )
                                return ex, lo

                            def emit_av(t, ex, lo):
                                st_g = 16 * b + t
                                for h in range(2):
                                    hg = 2 * slab + h
                                    nc.tensor.matmul(
                                        ps_av[64 * h : 64 * (h + 1), lo:],
                                        lhsT=v_sb[:, st_g, hg, :],
                                        rhs=ex[:, h, lo:],
                                        start=(t == 0),
                                        stop=(t == T - 1),
                                        tile_position=(0, 64 * h),
                                        skip_group_check=True,
                                    )
                                    nc.tensor.matmul(
                                        den_ps[64 * h : 64 * (h + 1), lo:],
                                        lhsT=allones,
                                        rhs=ex[:, h, lo:],
                                        start=(t == 0),
                                        stop=(t == T - 1),
                                        tile_position=(0, 64 * h),
                                        skip_group_check=True,
                                    )

                            # lookahead-1 software pipeline: scores(t+1) is
                            # emitted (and runs on PE) while exp(t) is on ACT
                            # lookahead-2 software pipeline
                            pend = []
                            for t in range(T):
                                pend.append((t, *emit_scores_exp(t)))
                                if len(pend) > 3:
                                    emit_av(*pend.pop(0))
                            for item in pend:
                                emit_av(*item)
                            den_rb = phd.tile([128, 512], fp32, tag="den_rb")
                            nc.vector.reciprocal(den_rb, den_ps)
                            nc.vector.tensor_mul(
                                aoT[:, slab, sq0 : sq0 + 512], ps_av, den_rb
                            )

            # --- phase C: output projection --------------------------------
            with tc.tile_pool(name="phc", bufs=3) as phc, tc.tile_pool(
                name="psC", bufs=2, space="PSUM"
            ) as psC:
                for st in range(NST if "C" in phases else 0):
                    ot = phc.tile([128, D_MODEL], fp32, tag="ot")
                    for eh in range(2):
                        ps = psC.tile([128, 512], fp32, tag="ps_o")
                        for s in range(NSLAB):
                            nc.tensor.matmul(
                                ps,
                                lhsT=aoT[:, s, st * 128 : (st + 1) * 128],
                                rhs=wo_sb[:, s, eh * 512 : (eh + 1) * 512],
                                start=(s == 0),
                                stop=(s == NSLAB - 1),
                            )
                        nc.vector.tensor_copy(ot[:, eh * 512 : (eh + 1) * 512], ps)
                    nc.sync.dma_start(
                        out=out[st * 128 : (st + 1) * 128, :], in_=ot
                    )
    _split_multi_waits(nc)
    return nc


def make_core_inputs(x, Wq, bq, Wk, bk, Wv, bv, Wo):
    """Host-side shard prep: returns list of 8 per-core input dicts."""
    x = np.asarray(x, F32)
    tri4 = np.zeros((128, 4, 512), BF16)
    for p in range(128):
        for j in range(4):
            tri4[p, j, 128 * j + p :] = 1.0
    ins = []
    for c in range(N_CORES):
        dp, tp = divmod(c, TP)
        es = slice(E * tp, E * (tp + 1))
        ins.append(
            {
                "x_bf": np.ascontiguousarray(
                    x[2 * dp : 2 * dp + 2].reshape(SL, D_MODEL)
                ).astype(BF16),
                "wq_t": np.ascontiguousarray(np.asarray(Wq, F32)[es, :].T).astype(BF16),
                "wk_t": np.ascontiguousarray(np.asarray(Wk, F32)[es, :].T).astype(BF16),
                "wv_t": np.ascontiguousarray(np.asarray(Wv, F32)[es, :].T).astype(BF16),
                "wo_t": np.ascontiguousarray(np.asarray(Wo, F32)[:, es].T).astype(BF16),
                "bqs": np.ascontiguousarray(
                    np.asarray(bq, F32)[es].reshape(NSLAB, 128).T
                ),
                "bks": np.ascontiguousarray(
                    np.asarray(bk, F32)[es].reshape(NSLAB, 128).T
                ),
                "bvs": np.ascontiguousarray(np.asarray(bv, F32)[es].reshape(1, E)),
                "tri4": tri4,
            }
        )
    return ins


def assemble_output(results, bo):
    """Sum TP partials per DP group, add output bias."""
    out = np.zeros((B, S, D_MODEL), F32)
    for dp in range(DP):
        acc = np.zeros((SL, D_MODEL), F32)
        for tp in range(TP):
            acc += results[dp * TP + tp]["out"]
        out[2 * dp : 2 * dp + 2] = acc.reshape(2, S, D_MODEL)
    return out + np.asarray(bo, F32)[None, None, :]


_EXEC_CACHE: dict = {}


def _build_exec(donate: bool):
    import jax
    from jax.experimental.shard_map import shard_map
    from jax.sharding import Mesh, PartitionSpec
    from concourse import bass2jax

    if "nc" in _EXEC_CACHE:
        nc = _EXEC_CACHE["nc"]
    else:
        nc = build_nc()
        _EXEC_CACHE["nc"] = nc
    bass2jax.install_neuronx_cc_hook()

    partition_name = nc.partition_id_tensor.name if nc.partition_id_tensor else None
    in_names, out_names, out_avals, zero_outs = [], [], [], []
    for alloc in nc.m.functions[0].allocations:
        if not isinstance(alloc, mybir.MemoryLocationSet):
            continue
        name = alloc.memorylocations[0].name
        if alloc.kind == "ExternalInput":
            if name != partition_name:
                in_names.append(name)
        elif alloc.kind == "ExternalOutput":
            out_names.append(name)
            shape = tuple(alloc.tensor_shape)
            dtype = mybir.dt.np(alloc.dtype)
            out_avals.append(jax.core.ShapedArray(shape, dtype))
            zero_outs.append(np.zeros(shape, dtype))
    n_params = len(in_names)
    n_outs = len(out_avals)
    all_names = in_names + out_names
    if partition_name is not None:
        all_names = all_names + [partition_name]

    def _body(*args):
        operands = list(args)
        if partition_name is not None:
            operands.append(bass2jax.partition_id_tensor())
        outs = bass2jax._bass_exec_p.bind(
            *operands,
            out_avals=tuple(out_avals),
            in_names=tuple(all_names),
            out_names=tuple(out_names),
            lowering_input_output_aliases=(),
            sim_require_finite=True,
            sim_require_nnan=True,
            nc=nc,
        )
        return tuple(outs)

    devices = jax.devices()[:N_CORES]
    mesh = Mesh(np.asarray(devices), ("core",))
    donate_nums = tuple(range(n_params, n_params + n_outs)) if donate else ()
    sharded = jax.jit(
        shard_map(
            _body,
            mesh=mesh,
            in_specs=(PartitionSpec("core"),) * (n_params + n_outs),
            out_specs=(PartitionSpec("core"),) * n_outs,
            check_rep=False,
        ),
        donate_argnums=donate_nums,
        keep_unused=True,
    )
    meta = (in_names, out_names, out_avals, zero_outs, n_params)
    return sharded, meta


def _get_runner():
    """Build (once) a cached jitted SPMD runner over 8 cores."""
    if "run" in _EXEC_CACHE:
        return _EXEC_CACHE["run"]

    sharded, meta = _build_exec(donate=True)
    in_names, out_names, out_avals, zero_outs, n_params = meta

    def run(in_maps):
        concat_in = [
            np.concatenate([np.asarray(m[name]) for m in in_maps], axis=0)
            for name in in_names
        ]
        concat_zeros = [
            np.zeros((N_CORES * z.shape[0], *z.shape[1:]), z.dtype) for z in zero_outs
        ]
        out_arrs = sharded(*concat_in, *concat_zeros)
        return [
            {
                name: np.asarray(out_arrs[i]).reshape(N_CORES, *out_avals[i].shape)[c]
                for i, name in enumerate(out_names)
            }
            for c in range(N_CORES)
        ]

    _EXEC_CACHE["run"] = run
    _EXEC_CACHE["sharded"] = sharded
    _EXEC_CACHE["meta"] = meta
    return run


def kernel(x, mask, Wq, bq, Wk, bk, Wv, bv, Wo, bo):
    run = _get_runner()
    ins = make_core_inputs(x, Wq, bq, Wk, bk, Wv, bv, Wo)
    results = run(ins)
    return assemble_output(results, bo)


# revision 71
# speedup vs baseline: 38.5644x; 1.0180x over previous
"""Trainium2 Bass kernel for nn_MultiHeadAttention_64733747085699.

Sharding: tensor-parallel over heads (4 heads / core) x data-parallel over
batch (2 batches / core) across 8 NeuronCores. Each core computes a partial
output projection for its 4 heads; the host sums the 4 TP partials per batch
group and adds the output bias.

Per-core device pipeline (all matmuls on PE, fp32 PSUM accumulation):
  xT  = transpose-load of x (bf16, via DMA xbar transpose)
  qT/kT [e,s] = W @ xT   (float32r out, per-partition bias in evacuation)
  v [s,e]    = xT.T @ Wv (bf16, bias via broadcast tile add)
  scoresT[sk,sq] = k^T q, f32r matmuls, two heads row-packed on the PE array
  expT = exp(scoresT/sqrt(d)) on ACT (scale fused), restricted to the causal
  column range; triangle mask applied as a 0/1 multiply on GpSimd
  attn_outT[d,sq] = v^T expT, two heads column-packed, lookahead-3 software
  pipeline against the ACT exp stream; softmax denominators accumulated
  partition-broadcast via an all-ones stationary matrix in the same pass
  normalization = DVE reciprocal + multiply during PSUM evacuation
  out[s,e] partial = attn_outT.T @ WoT
Host: bf16 cast + weight transposes + TP-partial reduction + biases.
"""
import sys

if "/opt/trn_rl_repo" not in sys.path:
    sys.path.insert(0, "/opt/trn_rl_repo")

import numpy as np
import ml_dtypes

import bass_rust
import concourse.bass as bass
import concourse.tile as tile
from concourse import mybir

BF16 = ml_dtypes.bfloat16
F32 = np.float32

D_MODEL = 1024
N_HEADS = 16
D_HEAD = 64
B, S = 4, 2048
N_CORES = 8
TP = 4          # head-parallel ranks
DP = 2          # batch-parallel groups
E = D_MODEL // TP        # 256 e-channels per core (4 heads)
SL = 2 * S               # 4096 core-local sequence rows (2 batches)
NK = D_MODEL // 128      # 8 contraction tiles
NSLAB = E // 128         # 2 e-slabs per core (2 heads each)
NST = SL // 128          # 32 local s-tiles
NSC = SL // 512          # 8 local s-chunks
SCALE = 1.0 / np.sqrt(D_HEAD)

fp32 = mybir.dt.float32
bf16 = mybir.dt.bfloat16


# ---------------------------------------------------------------------------
# Workaround: the pinned walrus codegen rejects instructions carrying more
# than one sync-wait command. After Tile scheduling, hoist extra waits onto
# same-engine NOPs inserted immediately before the offending instruction
# (semantically identical: the waits still complete before it executes).
def _split_multi_waits(nc: bass.Bass) -> None:
    for _, bbc in nc.bb_map.items():
        bb = bbc.bb
        insts = bb.instructions
        new_list = []
        changed = False
        for inst in insts:
            try:
                si = inst.sync_info
                waits = list(si.on_wait)
            except Exception:
                new_list.append(inst)
                continue
            if len(waits) > 1:
                changed = True
                for w in waits[:-1]:
                    nop = mybir.InstNoOp(
                        name=nc.get_next_instruction_name(), ins=[], outs=[]
                    )
                    nop.engine = inst.engine
                    nop.sync_info = bass_rust.SyncInfo(on_wait=[w], on_update=[])
                    nc.register_instruction(nop, overwrite=True)
                    new_list.append(nop)
                inst.sync_info = bass_rust.SyncInfo(
                    on_wait=[waits[-1]], on_update=list(si.on_update)
                )
            new_list.append(inst)
        if changed:
            bb.instructions = new_list
# ---------------------------------------------------------------------------


def build_nc(phases: str = "ABC", trace_sim: bool = False) -> bass.Bass:
    nc = bass.Bass("TRN2", target_bir_lowering=False, debug=False)
    x_bf = nc.dram_tensor("x_bf", [SL, D_MODEL], bf16, kind="ExternalInput")
    wq_t = nc.dram_tensor("wq_t", [D_MODEL, E], bf16, kind="ExternalInput")
    wk_t = nc.dram_tensor("wk_t", [D_MODEL, E], bf16, kind="ExternalInput")
    wv_t = nc.dram_tensor("wv_t", [D_MODEL, E], bf16, kind="ExternalInput")
    wo_t = nc.dram_tensor("wo_t", [E, D_MODEL], bf16, kind="ExternalInput")
    bqs = nc.dram_tensor("bqs", [128, NSLAB], fp32, kind="ExternalInput")
    bks = nc.dram_tensor("bks", [128, NSLAB], fp32, kind="ExternalInput")
    bvs = nc.dram_tensor("bvs", [1, E], fp32, kind="ExternalInput")
    tri4 = nc.dram_tensor("tri4", [128, 4, 512], bf16, kind="ExternalInput")
    out = nc.dram_tensor("out", [SL, D_MODEL], fp32, kind="ExternalOutput")

    with tile.TileContext(nc, trace_sim=trace_sim) as tc:
        with tc.tile_pool(name="persist", bufs=1) as persist:
            # --- persistent SBUF tensors -----------------------------------
            xTp = persist.tile([128, NK, SL], bf16, tag="xT")
            wk_sb = persist.tile([128, NK, E], bf16)
            nc.gpsimd.dma_start(out=wk_sb, in_=wk_t.rearrange("(k p) e -> p k e", p=128))
            bk_sb = persist.tile([128, NSLAB], fp32)
            nc.gpsimd.dma_start(out=bk_sb, in_=bks[:, :])
            if "X" in phases or "A" in phases:
                for k in range(NK):
                    nc.sync.dma_start_transpose(
                        out=xTp[:, k, 0 : SL // 2],
                        in_=x_bf[0 : SL // 2, k * 128 : (k + 1) * 128],
                    )
            wq_sb = persist.tile([128, NK, E], bf16)
            nc.gpsimd.dma_start(out=wq_sb, in_=wq_t.rearrange("(k p) e -> p k e", p=128))
            bq_sb = persist.tile([128, NSLAB], fp32)
            nc.gpsimd.dma_start(out=bq_sb, in_=bqs[:, :])
            wv_sb = persist.tile([128, NK, E], bf16)
            nc.gpsimd.dma_start(out=wv_sb, in_=wv_t.rearrange("(k p) e -> p k e", p=128))
            wo_sb = persist.tile([128, NSLAB, D_MODEL], bf16)
            nc.gpsimd.dma_start(out=wo_sb, in_=wo_t.rearrange("(s p) e -> p s e", p=128))
            tri_sb = persist.tile([128, 4, 512], bf16)
            nc.gpsimd.dma_start(out=tri_sb, in_=tri4[:, :, :])
            allones = persist.tile([128, D_HEAD], bf16)
            nc.vector.memset(allones, 1.0)
            # per-head bias-v broadcast tiles [128, 64] via partition-stride-0
            # SWDGE DMA from DRAM
            bv_b = persist.tile([128, 4, D_HEAD], fp32)
            for h in range(4):
                seg = bvs[0:1, h * D_HEAD : (h + 1) * D_HEAD]
                src = bass.AP(
                    tensor=seg.tensor,
                    offset=seg.offset,
                    ap=[[0, 128]] + list(seg.ap[1:]),
                )
                nc.gpsimd.dma_start(out=bv_b[:, h, :], in_=src)

            qT = persist.tile([128, NSLAB, SL], mybir.dt.float32r)
            kT = persist.tile([128, NSLAB, SL], mybir.dt.float32r)
            v_sb = persist.tile([128, NST, 4, D_HEAD], bf16)
            aoT = persist.tile([128, NSLAB, SL], bf16)

            # --- phase A: projections --------------------------------------
            with tc.tile_pool(name="pha", bufs=1) as pha, tc.tile_pool(
                name="psA", bufs=2, space="PSUM"
            ) as psA:
                xT = xTp
                if "X" in phases or "A" in phases:
                    lo = SL // 2
                    for k in range(NK):
                        nc.sync.dma_start_transpose(
                            out=xT[:, k, lo:],
                            in_=x_bf[lo:, k * 128 : (k + 1) * 128],
                        )

                # batch-0 projections first so attention can overlap batch-1's
                def emit_qk(wsb, bsb, dest, et, sc):
                    ps = psA.tile([128, 512], fp32, tag="ps_qk")
                    for k in range(NK):
                        nc.tensor.matmul(
                            ps,
                            lhsT=wsb[:, k, et * 128 : (et + 1) * 128],
                            rhs=xT[:, k, sc * 512 : (sc + 1) * 512],
                            start=(k == 0),
                            stop=(k == NK - 1),
                        )
                    nc.vector.tensor_scalar(
                        dest[:, et, sc * 512 : (sc + 1) * 512],
                        ps,
                        bsb[:, et : et + 1],
                        None,
                        mybir.AluOpType.add,
                    )

                def emit_v(st):
                    ps = psA.tile([128, E], fp32, tag="ps_v")
                    for k in range(NK):
                        nc.tensor.matmul(
                            ps,
                            lhsT=xT[:, k, st * 128 : (st + 1) * 128],
                            rhs=wv_sb[:, k, :],
                            start=(k == 0),
                            stop=(k == NK - 1),
                        )
                    for h in range(4):
                        nc.vector.tensor_add(
                            v_sb[:, st, h, :],
                            ps[:, h * D_HEAD : (h + 1) * D_HEAD],
                            bv_b[:, h, :],
                        )

                # batch-0 (half 0) first so its attention can overlap the
                # rest; within a half: slab-0 k/q, then v, then slab-1 k/q
                for half in range(2 if "A" in phases else 0):
                    scs = range(half * NSC // 2, (half + 1) * NSC // 2)
                    sts = range(half * NST // 2, (half + 1) * NST // 2)
                    for sc in scs:
                        emit_qk(wk_sb, bk_sb, kT, 0, sc)
                    for sc in scs:
                        emit_qk(wq_sb, bq_sb, qT, 0, sc)
                    for st in sts:
                        emit_v(st)
                    for sc in scs:
                        emit_qk(wk_sb, bk_sb, kT, 1, sc)
                    for sc in scs:
                        emit_qk(wq_sb, bq_sb, qT, 1, sc)

            # --- phase B: attention ----------------------------------------
            with tc.tile_pool(name="phb", bufs=6) as phb, tc.tile_pool(
                name="phd", bufs=2
            ) as phd, tc.tile_pool(name="psS", bufs=2, space="PSUM") as psS, tc.tile_pool(
                name="psAV", bufs=2, space="PSUM"
            ) as psAV, tc.tile_pool(name="psDen", bufs=2, space="PSUM") as psDen:
                for b in range(DP if "B" in phases else 0):
                    for slab in range(NSLAB):
                        for c in range(4):
                            sq0 = 2048 * b + 512 * c
                            T = 4 * c + 4
                            ps_av = psAV.tile([128, 512], fp32, tag="av")
                            den_ps = psDen.tile([128, 512], fp32, tag="dn")
                            def emit_scores_exp(t):
                                sk0 = 2048 * b + 128 * t
                                ps_s = psS.tile([128, 2, 512], fp32, tag="sc")
                                for h in range(2):
                                    nc.tensor.matmul(
                                        ps_s[:, h, :],
                                        lhsT=kT[
                                            64 * h : 64 * (h + 1),
                                            slab,
                                            sk0 : sk0 + 128,
                                        ],
                                        rhs=qT[
                                            64 * h : 64 * (h + 1), slab, sq0 : sq0 + 512
                                        ],
                                        start=True,
                                        stop=True,
                                        tile_position=(64 * h, 0),
                                    )
                                # columns < 128j are fully masked for this
                                # sk-tile: skip them in exp and downstream mms
                                j = t - 4 * c
                                lo = 128 * j if j > 0 else 0
                                ex = phb.tile([128, 2, 512], bf16, tag="ex")
                                nc.scalar.activation(
                                    out=ex[:, :, lo:],
                                    in_=ps_s[:, :, lo:],
                                    func=mybir.ActivationFunctionType.Exp,
                                    scale=float(SCALE),
                                )
                                if j >= 0:
                                    for h in range(2):
                                        nc.gpsimd.tensor_mul(
                                            ex[:, h, 128 * j : 128 * j + 128],
                                            ex[:, h, 128 * j : 128 * j + 128],
                                            tri_sb[:, j, 128 * j : 128 * j + 128],
                                        )
                                return ex, lo

                            def emit_av(t, ex, lo):
                                st_g = 16 * b + t
                                for h in range(2):
                                    hg = 2 * slab + h
                                    nc.tensor.matmul(
                                        ps_av[64 * h : 64 * (h + 1), lo:],
                                        lhsT=v_sb[:, st_g, hg, :],
                                        rhs=ex[:, h, lo:],
                                        start=(t == 0),
                                        stop=(t == T - 1),
                                        tile_position=(0, 64 * h),
                                        skip_group_check=True,
                                    )
                                    nc.tensor.matmul(
                                        den_ps[64 * h : 64 * (h + 1), lo:],
                                        lhsT=allones,
                                        rhs=ex[:, h, lo:],
                                        start=(t == 0),
                                        stop=(t == T - 1),
                                        tile_position=(0, 64 * h),
                                        skip_group_check=True,
                                    )

                            # lookahead-1 software pipeline: scores(t+1) is
                            # emitted (and runs on PE) while exp(t) is on ACT
                            # lookahead-2 software pipeline
                            pend = []
                            for t in range(T):
                                pend.append((t, *emit_scores_exp(t)))
                                if len(pend) > 3:
                                    emit_av(*pend.pop(0))
                            for item in pend:
                                emit_av(*item)
                            den_rb = phd.tile([128, 512], fp32, tag="den_rb")
                            nc.vector.reciprocal(den_rb, den_ps)
                            nc.vector.tensor_mul(
                                aoT[:, slab, sq0 : sq0 + 512], ps_av, den_rb
                            )

            # --- phase C: output projection --------------------------------
            with tc.tile_pool(name="phc", bufs=3) as phc, tc.tile_pool(
                name="psC", bufs=2, space="PSUM"
            ) as psC:
                for st in range(NST if "C" in phases else 0):
                    ot = phc.tile([128, D_MODEL], fp32, tag="ot")
                    for eh in range(2):
                        ps = psC.tile([128, 512], fp32, tag="ps_o")
                        for s in range(NSLAB):
                            nc.tensor.matmul(
                                ps,
                                lhsT=aoT[:, s, st * 128 : (st + 1) * 128],
                                rhs=wo_sb[:, s, eh * 512 : (eh + 1) * 512],
                                start=(s == 0),
                                stop=(s == NSLAB - 1),
                            )
                        if eh == 0:
                            nc.vector.tensor_copy(ot[:, 0:512], ps)
                        else:
                            nc.scalar.copy(ot[:, 512:1024], ps)
                    nc.sync.dma_start(
                        out=out[st * 128 : (st + 1) * 128, :], in_=ot
                    )
    _split_multi_waits(nc)
    return nc


def make_core_inputs(x, Wq, bq, Wk, bk, Wv, bv, Wo):
    """Host-side shard prep: returns list of 8 per-core input dicts."""
    x = np.asarray(x, F32)
    tri4 = np.zeros((128, 4, 512), BF16)
    for p in range(128):
        for j in range(4):
            tri4[p, j, 128 * j + p :] = 1.0
    ins = []
    for c in range(N_CORES):
        dp, tp = divmod(c, TP)
        es = slice(E * tp, E * (tp + 1))
        ins.append(
            {
                "x_bf": np.ascontiguousarray(
                    x[2 * dp : 2 * dp + 2].reshape(SL, D_MODEL)
                ).astype(BF16),
                "wq_t": np.ascontiguousarray(np.asarray(Wq, F32)[es, :].T).astype(BF16),
                "wk_t": np.ascontiguousarray(np.asarray(Wk, F32)[es, :].T).astype(BF16),
                "wv_t": np.ascontiguousarray(np.asarray(Wv, F32)[es, :].T).astype(BF16),
                "wo_t": np.ascontiguousarray(np.asarray(Wo, F32)[:, es].T).astype(BF16),
                "bqs": np.ascontiguousarray(
                    np.asarray(bq, F32)[es].reshape(NSLAB, 128).T
                ),
                "bks": np.ascontiguousarray(
                    np.asarray(bk, F32)[es].reshape(NSLAB, 128).T
                ),
                "bvs": np.ascontiguousarray(np.asarray(bv, F32)[es].reshape(1, E)),
                "tri4": tri4,
            }
        )
    return ins


def assemble_output(results, bo):
    """Sum TP partials per DP group, add output bias."""
    out = np.zeros((B, S, D_MODEL), F32)
    for dp in range(DP):
        acc = np.zeros((SL, D_MODEL), F32)
        for tp in range(TP):
            acc += results[dp * TP + tp]["out"]
        out[2 * dp : 2 * dp + 2] = acc.reshape(2, S, D_MODEL)
    return out + np.asarray(bo, F32)[None, None, :]


_EXEC_CACHE: dict = {}


def _build_exec(donate: bool):
    import jax
    from jax.experimental.shard_map import shard_map
    from jax.sharding import Mesh, PartitionSpec
    from concourse import bass2jax

    if "nc" in _EXEC_CACHE:
        nc = _EXEC_CACHE["nc"]
    else:
        nc = build_nc()
        _EXEC_CACHE["nc"] = nc
    bass2jax.install_neuronx_cc_hook()

    partition_name = nc.partition_id_tensor.name if nc.partition_id_tensor else None
    in_names, out_names, out_avals, zero_outs = [], [], [], []
    for alloc in nc.m.functions[0].allocations:
        if not isinstance(alloc, mybir.MemoryLocationSet):
            continue
        name = alloc.memorylocations[0].name
        if alloc.kind == "ExternalInput":
            if name != partition_name:
                in_names.append(name)
        elif alloc.kind == "ExternalOutput":
            out_names.append(name)
            shape = tuple(alloc.tensor_shape)
            dtype = mybir.dt.np(alloc.dtype)
            out_avals.append(jax.core.ShapedArray(shape, dtype))
            zero_outs.append(np.zeros(shape, dtype))
    n_params = len(in_names)
    n_outs = len(out_avals)
    all_names = in_names + out_names
    if partition_name is not None:
        all_names = all_names + [partition_name]

    def _body(*args):
        operands = list(args)
        if partition_name is not None:
            operands.append(bass2jax.partition_id_tensor())
        outs = bass2jax._bass_exec_p.bind(
            *operands,
            out_avals=tuple(out_avals),
            in_names=tuple(all_names),
            out_names=tuple(out_names),
            lowering_input_output_aliases=(),
            sim_require_finite=True,
            sim_require_nnan=True,
            nc=nc,
        )
        return tuple(outs)

    devices = jax.devices()[:N_CORES]
    mesh = Mesh(np.asarray(devices), ("core",))
    donate_nums = tuple(range(n_params, n_params + n_outs)) if donate else ()
    sharded = jax.jit(
        shard_map(
            _body,
            mesh=mesh,
            in_specs=(PartitionSpec("core"),) * (n_params + n_outs),
            out_specs=(PartitionSpec("core"),) * n_outs,
            check_rep=False,
        ),
        donate_argnums=donate_nums,
        keep_unused=True,
    )
    meta = (in_names, out_names, out_avals, zero_outs, n_params)
    return sharded, meta


def _get_runner():
    """Build (once) a cached jitted SPMD runner over 8 cores."""
    if "run" in _EXEC_CACHE:
        return _EXEC_CACHE["run"]

    sharded, meta = _build_exec(donate=True)
    in_names, out_names, out_avals, zero_outs, n_params = meta

    def run(in_maps):
        concat_in = [
            np.concatenate([np.asarray(m[name]) for m in in_maps], axis=0)
            for name in in_names
        ]
        concat_zeros = [
            np.zeros((N_CORES * z.shape[0], *z.shape[1:]), z.dtype) for z in zero_outs
        ]
        out_arrs = sharded(*concat_in, *concat_zeros)
        return [
            {
                name: np.asarray(out_arrs[i]).reshape(N_CORES, *out_avals[i].shape)[c]
                for i, name in enumerate(out_names)
            }
            for c in range(N_CORES)
        ]

    _EXEC_CACHE["run"] = run
    _EXEC_CACHE["sharded"] = sharded
    _EXEC_CACHE["meta"] = meta
    return run


def kernel(x, mask, Wq, bq, Wk, bk, Wv, bv, Wo, bo):
    run = _get_runner()
    ins = make_core_inputs(x, Wq, bq, Wk, bk, Wv, bv, Wo)
    results = run(ins)
    return assemble_output(results, bo)


# revision 73
# speedup vs baseline: 38.8809x; 1.0082x over previous
"""Trainium2 Bass kernel for nn_MultiHeadAttention_64733747085699.

Sharding: tensor-parallel over heads (4 heads / core) x data-parallel over
batch (2 batches / core) across 8 NeuronCores. Each core computes a partial
output projection for its 4 heads; the host sums the 4 TP partials per batch
group and adds the output bias.

Per-core device pipeline (all matmuls on PE, fp32 PSUM accumulation):
  xT  = transpose-load of x (bf16, via DMA xbar transpose)
  qT/kT [e,s] = W @ xT   (float32r out, per-partition bias in evacuation)
  v [s,e]    = xT.T @ Wv (bf16, bias via broadcast tile add)
  scoresT[sk,sq] = k^T q, f32r matmuls, two heads row-packed on the PE array
  expT = exp(scoresT/sqrt(d)) on ACT (scale fused), restricted to the causal
  column range; triangle mask applied as a 0/1 multiply on GpSimd
  attn_outT[d,sq] = v^T expT, two heads column-packed, lookahead-3 software
  pipeline against the ACT exp stream; softmax denominators accumulated
  partition-broadcast via an all-ones stationary matrix in the same pass
  normalization = DVE reciprocal + multiply during PSUM evacuation
  out[s,e] partial = attn_outT.T @ WoT
Host: bf16 cast + weight transposes + TP-partial reduction + biases.
"""
import sys

if "/opt/trn_rl_repo" not in sys.path:
    sys.path.insert(0, "/opt/trn_rl_repo")

import numpy as np
import ml_dtypes

import bass_rust
import concourse.bass as bass
import concourse.tile as tile
from concourse import mybir

BF16 = ml_dtypes.bfloat16
F32 = np.float32

D_MODEL = 1024
N_HEADS = 16
D_HEAD = 64
B, S = 4, 2048
N_CORES = 8
TP = 4          # head-parallel ranks
DP = 2          # batch-parallel groups
E = D_MODEL // TP        # 256 e-channels per core (4 heads)
SL = 2 * S               # 4096 core-local sequence rows (2 batches)
NK = D_MODEL // 128      # 8 contraction tiles
NSLAB = E // 128         # 2 e-slabs per core (2 heads each)
NST = SL // 128          # 32 local s-tiles
NSC = SL // 512          # 8 local s-chunks
SCALE = 1.0 / np.sqrt(D_HEAD)

fp32 = mybir.dt.float32
bf16 = mybir.dt.bfloat16


# ---------------------------------------------------------------------------
# Workaround: the pinned walrus codegen rejects instructions carrying more
# than one sync-wait command. After Tile scheduling, hoist extra waits onto
# same-engine NOPs inserted immediately before the offending instruction
# (semantically identical: the waits still complete before it executes).
def _split_multi_waits(nc: bass.Bass) -> None:
    for _, bbc in nc.bb_map.items():
        bb = bbc.bb
        insts = bb.instructions
        new_list = []
        changed = False
        for inst in insts:
            try:
                si = inst.sync_info
                waits = list(si.on_wait)
            except Exception:
                new_list.append(inst)
                continue
            if len(waits) > 1:
                changed = True
                for w in waits[:-1]:
                    nop = mybir.InstNoOp(
                        name=nc.get_next_instruction_name(), ins=[], outs=[]
                    )
                    nop.engine = inst.engine
                    nop.sync_info = bass_rust.SyncInfo(on_wait=[w], on_update=[])
                    nc.register_instruction(nop, overwrite=True)
                    new_list.append(nop)
                inst.sync_info = bass_rust.SyncInfo(
                    on_wait=[waits[-1]], on_update=list(si.on_update)
                )
            new_list.append(inst)
        if changed:
            bb.instructions = new_list
# ---------------------------------------------------------------------------


def build_nc(phases: str = "ABC", trace_sim: bool = False) -> bass.Bass:
    nc = bass.Bass("TRN2", target_bir_lowering=False, debug=False)
    x_bf = nc.dram_tensor("x_bf", [SL, D_MODEL], bf16, kind="ExternalInput")
    wq_t = nc.dram_tensor("wq_t", [D_MODEL, E], bf16, kind="ExternalInput")
    wk_t = nc.dram_tensor("wk_t", [D_MODEL, E], bf16, kind="ExternalInput")
    wv_t = nc.dram_tensor("wv_t", [D_MODEL, E], bf16, kind="ExternalInput")
    wo_t = nc.dram_tensor("wo_t", [E, D_MODEL], bf16, kind="ExternalInput")
    bqs = nc.dram_tensor("bqs", [128, NSLAB], fp32, kind="ExternalInput")
    bks = nc.dram_tensor("bks", [128, NSLAB], fp32, kind="ExternalInput")
    bvs = nc.dram_tensor("bvs", [1, E], fp32, kind="ExternalInput")
    tri4 = nc.dram_tensor("tri4", [128, 4, 512], bf16, kind="ExternalInput")
    out = nc.dram_tensor("out", [SL, D_MODEL], fp32, kind="ExternalOutput")

    with tile.TileContext(nc, trace_sim=trace_sim) as tc:
        with tc.tile_pool(name="persist", bufs=1) as persist:
            # --- persistent SBUF tensors -----------------------------------
            xTp = persist.tile([128, NK, SL], bf16, tag="xT")
            wk_sb = persist.tile([128, NK, E], bf16)
            nc.gpsimd.dma_start(out=wk_sb, in_=wk_t.rearrange("(k p) e -> p k e", p=128))
            bk_sb = persist.tile([128, NSLAB], fp32)
            nc.gpsimd.dma_start(out=bk_sb, in_=bks[:, :])
            if "X" in phases or "A" in phases:
                for k in range(NK):
                    nc.sync.dma_start_transpose(
                        out=xTp[:, k, 0 : SL // 2],
                        in_=x_bf[0 : SL // 2, k * 128 : (k + 1) * 128],
                    )
            wq_sb = persist.tile([128, NK, E], bf16)
            nc.gpsimd.dma_start(out=wq_sb, in_=wq_t.rearrange("(k p) e -> p k e", p=128))
            bq_sb = persist.tile([128, NSLAB], fp32)
            nc.gpsimd.dma_start(out=bq_sb, in_=bqs[:, :])
            wv_sb = persist.tile([128, NK, E], bf16)
            nc.gpsimd.dma_start(out=wv_sb, in_=wv_t.rearrange("(k p) e -> p k e", p=128))
            wo_sb = persist.tile([128, NSLAB, D_MODEL], bf16)
            nc.gpsimd.dma_start(out=wo_sb, in_=wo_t.rearrange("(s p) e -> p s e", p=128))
            tri_sb = persist.tile([128, 4, 512], bf16)
            nc.gpsimd.dma_start(out=tri_sb, in_=tri4[:, :, :])
            allones = persist.tile([128, D_HEAD], bf16)
            nc.vector.memset(allones, 1.0)
            # per-head bias-v broadcast tiles [128, 64] via partition-stride-0
            # SWDGE DMA from DRAM
            bv_b = persist.tile([128, 4, D_HEAD], fp32)
            for h in range(4):
                seg = bvs[0:1, h * D_HEAD : (h + 1) * D_HEAD]
                src = bass.AP(
                    tensor=seg.tensor,
                    offset=seg.offset,
                    ap=[[0, 128]] + list(seg.ap[1:]),
                )
                nc.gpsimd.dma_start(out=bv_b[:, h, :], in_=src)

            qT = persist.tile([128, NSLAB, SL], mybir.dt.float32r)
            kT = persist.tile([128, NSLAB, SL], mybir.dt.float32r)
            v_sb = persist.tile([128, NST, 4, D_HEAD], bf16)
            aoT = persist.tile([128, NSLAB, SL], bf16)

            # --- phase A: projections --------------------------------------
            with tc.tile_pool(name="pha", bufs=1) as pha, tc.tile_pool(
                name="psA", bufs=4, space="PSUM"
            ) as psA:
                xT = xTp
                if "X" in phases or "A" in phases:
                    lo = SL // 2
                    for k in range(NK):
                        nc.sync.dma_start_transpose(
                            out=xT[:, k, lo:],
                            in_=x_bf[lo:, k * 128 : (k + 1) * 128],
                        )

                # batch-0 projections first so attention can overlap batch-1's
                def emit_qk(wsb, bsb, dest, et, sc):
                    ps = psA.tile([128, 512], fp32, tag="ps_qk")
                    for k in range(NK):
                        nc.tensor.matmul(
                            ps,
                            lhsT=wsb[:, k, et * 128 : (et + 1) * 128],
                            rhs=xT[:, k, sc * 512 : (sc + 1) * 512],
                            start=(k == 0),
                            stop=(k == NK - 1),
                        )
                    nc.vector.tensor_scalar(
                        dest[:, et, sc * 512 : (sc + 1) * 512],
                        ps,
                        bsb[:, et : et + 1],
                        None,
                        mybir.AluOpType.add,
                    )

                def emit_v(st):
                    ps = psA.tile([128, E], fp32, tag="ps_v")
                    for k in range(NK):
                        nc.tensor.matmul(
                            ps,
                            lhsT=xT[:, k, st * 128 : (st + 1) * 128],
                            rhs=wv_sb[:, k, :],
                            start=(k == 0),
                            stop=(k == NK - 1),
                        )
                    for h in range(4):
                        nc.vector.tensor_add(
                            v_sb[:, st, h, :],
                            ps[:, h * D_HEAD : (h + 1) * D_HEAD],
                            bv_b[:, h, :],
                        )

                # batch-0 (half 0) first so its attention can overlap the
                # rest; within a half: slab-0 k/q, then v, then slab-1 k/q
                for half in range(2 if "A" in phases else 0):
                    scs = range(half * NSC // 2, (half + 1) * NSC // 2)
                    sts = range(half * NST // 2, (half + 1) * NST // 2)
                    for sc in scs:
                        emit_qk(wk_sb, bk_sb, kT, 0, sc)
                    for sc in scs:
                        emit_qk(wq_sb, bq_sb, qT, 0, sc)
                    for st in sts:
                        emit_v(st)
                    for sc in scs:
                        emit_qk(wk_sb, bk_sb, kT, 1, sc)
                    for sc in scs:
                        emit_qk(wq_sb, bq_sb, qT, 1, sc)

            # --- phase B: attention ----------------------------------------
            with tc.tile_pool(name="phb", bufs=8) as phb, tc.tile_pool(
                name="phd", bufs=3
            ) as phd, tc.tile_pool(name="psS", bufs=2, space="PSUM") as psS, tc.tile_pool(
                name="psAV", bufs=2, space="PSUM"
            ) as psAV, tc.tile_pool(name="psDen", bufs=2, space="PSUM") as psDen:
                for b in range(DP if "B" in phases else 0):
                    for slab in range(NSLAB):
                        for c in range(4):
                            sq0 = 2048 * b + 512 * c
                            T = 4 * c + 4
                            ps_av = psAV.tile([128, 512], fp32, tag="av")
                            den_ps = psDen.tile([128, 512], fp32, tag="dn")
                            def emit_scores_exp(t):
                                sk0 = 2048 * b + 128 * t
                                ps_s = psS.tile([128, 2, 512], fp32, tag="sc")
                                for h in range(2):
                                    nc.tensor.matmul(
                                        ps_s[:, h, :],
                                        lhsT=kT[
                                            64 * h : 64 * (h + 1),
                                            slab,
                                            sk0 : sk0 + 128,
                                        ],
                                        rhs=qT[
                                            64 * h : 64 * (h + 1), slab, sq0 : sq0 + 512
                                        ],
                                        start=True,
                                        stop=True,
                                        tile_position=(64 * h, 0),
                                    )
                                # columns < 128j are fully masked for this
                                # sk-tile: skip them in exp and downstream mms
                                j = t - 4 * c
                                lo = 128 * j if j > 0 else 0
                                ex = phb.tile([128, 2, 512], bf16, tag="ex")
                                nc.scalar.activation(
                                    out=ex[:, :, lo:],
                                    in_=ps_s[:, :, lo:],
                                    func=mybir.ActivationFunctionType.Exp,
                                    scale=float(SCALE),
                                )
                                if j >= 0:
                                    for h in range(2):
                                        nc.gpsimd.tensor_mul(
                                            ex[:, h, 128 * j : 128 * j + 128],
                                            ex[:, h, 128 * j : 128 * j + 128],
                                            tri_sb[:, j, 128 * j : 128 * j + 128],
                                        )
                                return ex, lo

                            def emit_av(t, ex, lo):
                                st_g = 16 * b + t
                                for h in range(2):
                                    hg = 2 * slab + h
                                    nc.tensor.matmul(
                                        ps_av[64 * h : 64 * (h + 1), lo:],
                                        lhsT=v_sb[:, st_g, hg, :],
                                        rhs=ex[:, h, lo:],
                                        start=(t == 0),
                                        stop=(t == T - 1),
                                        tile_position=(0, 64 * h),
                                        skip_group_check=True,
                                    )
                                    nc.tensor.matmul(
                                        den_ps[64 * h : 64 * (h + 1), lo:],
                                        lhsT=allones,
                                        rhs=ex[:, h, lo:],
                                        start=(t == 0),
                                        stop=(t == T - 1),
                                        tile_position=(0, 64 * h),
                                        skip_group_check=True,
                                    )

                            # lookahead-1 software pipeline: scores(t+1) is
                            # emitted (and runs on PE) while exp(t) is on ACT
                            # lookahead-2 software pipeline
                            pend = []
                            for t in range(T):
                                pend.append((t, *emit_scores_exp(t)))
                                if len(pend) > 3:
                                    emit_av(*pend.pop(0))
                            for item in pend:
                                emit_av(*item)
                            den_rb = phd.tile([128, 512], fp32, tag="den_rb")
                            nc.vector.reciprocal(den_rb, den_ps)
                            nc.vector.tensor_mul(
                                aoT[:, slab, sq0 : sq0 + 512], ps_av, den_rb
                            )

            # --- phase C: output projection --------------------------------
            with tc.tile_pool(name="phc", bufs=3) as phc, tc.tile_pool(
                name="psC", bufs=2, space="PSUM"
            ) as psC:
                for st in range(NST if "C" in phases else 0):
                    ot = phc.tile([128, D_MODEL], fp32, tag="ot")
                    for eh in range(2):
                        ps = psC.tile([128, 512], fp32, tag="ps_o")
                        for s in range(NSLAB):
                            nc.tensor.matmul(
                                ps,
                                lhsT=aoT[:, s, st * 128 : (st + 1) * 128],
                                rhs=wo_sb[:, s, eh * 512 : (eh + 1) * 512],
                                start=(s == 0),
                                stop=(s == NSLAB - 1),
                            )
                        if eh == 0:
                            nc.vector.tensor_copy(ot[:, 0:512], ps)
                        else:
                            nc.scalar.copy(ot[:, 512:1024], ps)
                    nc.sync.dma_start(
                        out=out[st * 128 : (st + 1) * 128, :], in_=ot
                    )
    _split_multi_waits(nc)
    return nc


def make_core_inputs(x, Wq, bq, Wk, bk, Wv, bv, Wo):
    """Host-side shard prep: returns list of 8 per-core input dicts."""
    x = np.asarray(x, F32)
    tri4 = np.zeros((128, 4, 512), BF16)
    for p in range(128):
        for j in range(4):
            tri4[p, j, 128 * j + p :] = 1.0
    ins = []
    for c in range(N_CORES):
        dp, tp = divmod(c, TP)
        es = slice(E * tp, E * (tp + 1))
        ins.append(
            {
                "x_bf": np.ascontiguousarray(
                    x[2 * dp : 2 * dp + 2].reshape(SL, D_MODEL)
                ).astype(BF16),
                "wq_t": np.ascontiguousarray(np.asarray(Wq, F32)[es, :].T).astype(BF16),
                "wk_t": np.ascontiguousarray(np.asarray(Wk, F32)[es, :].T).astype(BF16),
                "wv_t": np.ascontiguousarray(np.asarray(Wv, F32)[es, :].T).astype(BF16),
                "wo_t": np.ascontiguousarray(np.asarray(Wo, F32)[:, es].T).astype(BF16),
                "bqs": np.ascontiguousarray(
                    np.asarray(bq, F32)[es].reshape(NSLAB, 128).T
                ),
                "bks": np.ascontiguousarray(
                    np.asarray(bk, F32)[es].reshape(NSLAB, 128).T
                ),
                "bvs": np.ascontiguousarray(np.asarray(bv, F32)[es].reshape(1, E)),
                "tri4": tri4,
            }
        )
    return ins


def assemble_output(results, bo):
    """Sum TP partials per DP group, add output bias."""
    out = np.zeros((B, S, D_MODEL), F32)
    for dp in range(DP):
        acc = np.zeros((SL, D_MODEL), F32)
        for tp in range(TP):
            acc += results[dp * TP + tp]["out"]
        out[2 * dp : 2 * dp + 2] = acc.reshape(2, S, D_MODEL)
    return out + np.asarray(bo, F32)[None, None, :]


_EXEC_CACHE: dict = {}


def _build_exec(donate: bool):
    import jax
    from jax.experimental.shard_map import shard_map
    from jax.sharding import Mesh, PartitionSpec
    from concourse import bass2jax

    if "nc" in _EXEC_CACHE:
        nc = _EXEC_CACHE["nc"]
    else:
        nc = build_nc()
        _EXEC_CACHE["nc"] = nc
    bass2jax.install_neuronx_cc_hook()

    partition_name = nc.partition_id_tensor.name if nc.partition_id_tensor else None
    in_names, out_names, out_avals, zero_outs = [], [], [], []
    for alloc in nc.m.functions[0].allocations:
        if not isinstance(alloc, mybir.MemoryLocationSet):
            continue
        name = alloc.memorylocations[0].name
        if alloc.kind == "ExternalInput":
            if name != partition_name:
                in_names.append(name)
        elif alloc.kind == "ExternalOutput":
            out_names.append(name)
            shape = tuple(alloc.tensor_shape)
            dtype = mybir.dt.np(alloc.dtype)
            out_avals.append(jax.core.ShapedArray(shape, dtype))
            zero_outs.append(np.zeros(shape, dtype))
    n_params = len(in_names)
    n_outs = len(out_avals)
    all_names = in_names + out_names
    if partition_name is not None:
        all_names = all_names + [partition_name]

    def _body(*args):
        operands = list(args)
        if partition_name is not None:
            operands.append(bass2jax.partition_id_tensor())
        outs = bass2jax._bass_exec_p.bind(
            *operands,
            out_avals=tuple(out_avals),
            in_names=tuple(all_names),
            out_names=tuple(out_names),
            lowering_input_output_aliases=(),
            sim_require_finite=True,
            sim_require_nnan=True,
            nc=nc,
        )
        return tuple(outs)

    devices = jax.devices()[:N_CORES]
    mesh = Mesh(np.asarray(devices), ("core",))
    donate_nums = tuple(range(n_params, n_params + n_outs)) if donate else ()
    sharded = jax.jit(
        shard_map(
            _body,
            mesh=mesh,
            in_specs=(PartitionSpec("core"),) * (n_params + n_outs),
            out_specs=(PartitionSpec("core"),) * n_outs,
            check_rep=False,
        ),
        donate_argnums=donate_nums,
        keep_unused=True,
    )
    meta = (in_names, out_names, out_avals, zero_outs, n_params)
    return sharded, meta


def _get_runner():
    """Build (once) a cached jitted SPMD runner over 8 cores."""
    if "run" in _EXEC_CACHE:
        return _EXEC_CACHE["run"]

    sharded, meta = _build_exec(donate=True)
    in_names, out_names, out_avals, zero_outs, n_params = meta

    def run(in_maps):
        concat_in = [
            np.concatenate([np.asarray(m[name]) for m in in_maps], axis=0)
            for name in in_names
        ]
        concat_zeros = [
            np.zeros((N_CORES * z.shape[0], *z.shape[1:]), z.dtype) for z in zero_outs
        ]
        out_arrs = sharded(*concat_in, *concat_zeros)
        return [
            {
                name: np.asarray(out_arrs[i]).reshape(N_CORES, *out_avals[i].shape)[c]
                for i, name in enumerate(out_names)
            }
            for c in range(N_CORES)
        ]

    _EXEC_CACHE["run"] = run
    _EXEC_CACHE["sharded"] = sharded
    _EXEC_CACHE["meta"] = meta
    return run


def kernel(x, mask, Wq, bq, Wk, bk, Wv, bv, Wo, bo):
    run = _get_runner()
    ins = make_core_inputs(x, Wq, bq, Wk, bk, Wv, bv, Wo)
    results = run(ins)
    return assemble_output(results, bo)
